# revision 7
# baseline (speedup 1.0000x reference)
"""Trainium2 Bass kernel for nn_CosineLoss (cosine-similarity pseudo-label CE loss).

Data-parallel over the flattened (B*P) patch dimension across 8 NeuronCores.

Per core the device computes, for each patch x (row of features):
  q_c  = dot(x, a_c / ||a_c||)   for the 4 prototypes   (PE, fp32r matmuls)
  n2   = ||x||^2                 (squares on ACT/DVE -> bf16, reduced on PE
                                  against a ones vector)
  keep = (q_0 > q_l) & (q_0 > 0) & (q_0^2 > 0.36 * n2)    [== sim_back>sim_sea
                                                           & sim_back>0.6]
  pseudo = is_foreground & ~keep
  s    = softmax(z); lse2 = log(sum(exp(s)))            (double-softmax CE)
  pp   = pseudo ? w_l*(lse2-s_l) : w_0*(lse2-s_0)       (masked for padding)
and returns per-partition partial sums of pp; the host adds them up and
divides by B*P.

Features are supplied to the device pre-transposed ([D, rows] per core) so the
contraction dim D lands on SBUF partitions; everything else is index prep on
tiny tensors.
"""

import numpy as np
from contextlib import ExitStack

import concourse.bass as bass
import concourse.bacc as bacc
import concourse.tile as tile
from concourse import mybir
from concourse.bass_utils import run_bass_kernel_spmd

# Problem constants (hardcoded; kernel.py must be self-contained).
B, P, D, C = 512, 45, 2048, 4
EPS = 1e-8
THRESH2 = 0.36  # THRESH**2, THRESH = 0.6
NCORES = 8
ROWS = B * P                 # 23040 patches
RT = 23                      # row tiles of 128 per core
R = RT * 128                 # 2944 padded rows per core
K = D // 128                 # 16 contraction chunks
GROUPS = [(0, 512), (512, 512), (1024, 512), (1536, 512), (2048, 512), (2560, 384)]
ACT_SQUARE_CHUNKS = 12       # chunks 0..11 squared on ScalarE, rest on VectorE

F32 = mybir.dt.float32
F32R = mybir.dt.float32r
BF16 = mybir.dt.bfloat16

_CACHE = {}


def _build():
    nc = bacc.Bacc("TRN2", target_bir_lowering=False, debug=False)
    featt = nc.dram_tensor("featt", [D, R], F32R, kind="ExternalInput").ap()
    avgtn = nc.dram_tensor("avgtn", [128, K * C], F32R, kind="ExternalInput").ap()
    zrow = nc.dram_tensor("zrow", [128, RT * C], F32, kind="ExternalInput").ap()
    meta = nc.dram_tensor("meta", [128, RT * 8], F32, kind="ExternalInput").ap()
    eye5 = nc.dram_tensor("eye5", [5, 5], F32, kind="ExternalInput").ap()
    out = nc.dram_tensor("out", [128, 1], F32, kind="ExternalOutput").ap()

    with tile.TileContext(nc) as tc, ExitStack() as ctx:
        consts = ctx.enter_context(tc.tile_pool(name="consts", bufs=1))
        gpool = ctx.enter_context(tc.tile_pool(name="gpool", bufs=2))
        sqpool = ctx.enter_context(tc.tile_pool(name="sqpool", bufs=2))
        sb = ctx.enter_context(tc.tile_pool(name="sb", bufs=1))
        qps = ctx.enter_context(tc.tile_pool(name="qps", bufs=2, space="PSUM"))
        nps = ctx.enter_context(tc.tile_pool(name="nps", bufs=2, space="PSUM"))
        tps = ctx.enter_context(tc.tile_pool(name="tps", bufs=1, space="PSUM"))

        # Constants / small inputs
        avgtn_sb = consts.tile([128, K, C], F32R)
        nc.sync.dma_start(out=avgtn_sb, in_=avgtn.rearrange("p (k c) -> p k c", c=C))
        eye5_sb = consts.tile([5, 5], F32)
        nc.sync.dma_start(out=eye5_sb, in_=eye5)
        eye4 = eye5_sb[0:4, 0:4]
        eye1 = eye5_sb[0:1, 0:1]
        ones_sb = consts.tile([128, 1], BF16)
        nc.vector.memset(ones_sb, 1.0)
        zsb = sb.tile([128, RT, C], F32)
        nc.sync.dma_start(out=zsb, in_=zrow.rearrange("p (t c) -> p t c", c=C))
        msb = sb.tile([128, RT, 8], F32)
        nc.sync.dma_start(out=msb, in_=meta.rearrange("p (t c) -> p t c", c=8))

        stq = sb.tile([4, R], F32)
        stn = sb.tile([1, R], F32)
        featt_r = featt.rearrange("(k p) r -> p k r", p=128)

        for off, w in GROUPS:
            g = gpool.tile([128, K, w], F32R, tag="g")
            nc.sync.dma_start(out=g, in_=featt_r[:, :, off:off + w])
            sq = sqpool.tile([128, K, w], BF16, tag="sq")
            # squares (bf16 out); 4-chunk ops, split ACT/DVE
            for k0 in range(0, K, 4):
                src = g[:, k0:k0 + 4, :].bitcast(F32)
                dst = sq[:, k0:k0 + 4, :]
                if k0 < ACT_SQUARE_CHUNKS:
                    nc.scalar.activation(dst, src, mybir.ActivationFunctionType.Square)
                else:
                    nc.vector.tensor_mul(dst, src, src)
            pq = qps.tile([C, w], F32, tag="pq")
            for k in range(K):
                nc.tensor.matmul(pq, avgtn_sb[:, k, :], g[:, k, :],
                                 start=(k == 0), stop=(k == K - 1))
            pn = nps.tile([1, w], F32, tag="pn")
            for k in range(K):
                nc.tensor.matmul(pn, ones_sb, sq[:, k, :],
                                 start=(k == 0), stop=(k == K - 1))
            nc.vector.tensor_copy(stq[:, off:off + w], pq)
            nc.vector.tensor_copy(stn[:, off:off + w], pn)

        # Transpose stq [4, R] -> qn [128, RT, 4] and stn [1, R] -> n2t [128, RT]
        # via PE (2x23 small transposes)
        ptq = tps.tile([128, RT * 4], F32)
        ptn = tps.tile([128, RT], F32)
        for t in range(RT):
            nc.tensor.transpose(ptq[:, t * 4:(t + 1) * 4],
                                stq[:, t * 128:(t + 1) * 128], eye4)
            nc.tensor.transpose(ptn[:, t:t + 1],
                                stn[:, t * 128:(t + 1) * 128], eye1)
        qn = sb.tile([128, RT, 4], F32)
        nc.vector.tensor_copy(qn.rearrange("p t c -> p (t c)"), ptq)
        n2t = sb.tile([128, RT], F32)
        nc.vector.tensor_copy(n2t, ptn)

        # ---- epilogue (all [128, RT(,C)] shaped) ----
        _tc = [0]

        def t23():
            _tc[0] += 1
            return sb.tile([128, RT], F32, name=f"t23_{_tc[0]}", tag=f"t23_{_tc[0]}")

        oh = msb[:, :, 0:4]
        wl = msb[:, :, 4]
        fgv = msb[:, :, 5]
        w0v = msb[:, :, 6]
        q0 = qn[:, :, 0]
        n2 = n2t

        e = sb.tile([128, RT, C], F32)
        nc.scalar.activation(e, zsb, mybir.ActivationFunctionType.Exp)
        zsum = t23()
        nc.vector.reduce_sum(zsum, e, axis=mybir.AxisListType.X)
        rz = t23()
        nc.vector.reciprocal(rz, zsum)
        s = sb.tile([128, RT, C], F32)
        nc.vector.tensor_mul(s, e, rz.unsqueeze(2).broadcast_to([128, RT, C]))
        es = sb.tile([128, RT, C], F32)
        nc.scalar.activation(es, s, mybir.ActivationFunctionType.Exp)
        essum = t23()
        nc.vector.reduce_sum(essum, es, axis=mybir.AxisListType.X)
        lse2 = t23()
        nc.scalar.activation(lse2, essum, mybir.ActivationFunctionType.Ln)

        soh = sb.tile([128, RT, C], F32)
        nc.vector.tensor_mul(soh, s, oh)
        sl = t23()
        nc.vector.reduce_sum(sl, soh, axis=mybir.AxisListType.X)
        qoh = sb.tile([128, RT, C], F32)
        nc.vector.tensor_mul(qoh, qn[:, :, 0:4], oh)
        ql = t23()
        nc.vector.reduce_sum(ql, qoh, axis=mybir.AxisListType.X)

        c1 = t23()
        nc.vector.tensor_tensor(c1, q0, ql, op=mybir.AluOpType.is_gt)
        q0sq = t23()
        nc.vector.tensor_mul(q0sq, q0, q0)
        t2 = t23()
        nc.vector.tensor_scalar_mul(t2, n2, THRESH2)
        c2a = t23()
        nc.vector.tensor_scalar(c2a, q0, 0.0, None, op0=mybir.AluOpType.is_gt)
        c2b = t23()
        nc.vector.tensor_tensor(c2b, q0sq, t2, op=mybir.AluOpType.is_gt)
        keep = t23()
        nc.vector.tensor_mul(keep, c1, c2a)
        keep2 = t23()
        nc.vector.tensor_mul(keep2, keep, c2b)
        # pv = fgv * (1 - keep2) = fgv - fgv*keep2
        fk = t23()
        nc.vector.tensor_mul(fk, fgv, keep2)
        pv = t23()
        nc.vector.tensor_sub(pv, fgv, fk)

        base = t23()
        nc.vector.tensor_sub(base, lse2, s[:, :, 0])
        alt = t23()
        nc.vector.tensor_sub(alt, lse2, sl)
        b1 = t23()
        nc.vector.tensor_mul(b1, w0v, base)
        a1 = t23()
        nc.vector.tensor_mul(a1, wl, alt)
        dd = t23()
        nc.vector.tensor_sub(dd, a1, b1)
        t3 = t23()
        nc.vector.tensor_mul(t3, pv, dd)
        pp = t23()
        nc.vector.tensor_add(pp, t3, b1)

        rowsum = sb.tile([128, 1], F32)
        nc.vector.reduce_sum(rowsum, pp, axis=mybir.AxisListType.X)
        nc.sync.dma_start(out=out, in_=rowsum)

    nc.compile()
    return nc


def _prep(features, average_features, outputs, labels_onehot, weights):
    feats = np.ascontiguousarray(np.asarray(features, np.float32).reshape(ROWS, D))
    z = np.asarray(outputs, np.float32).reshape(ROWS, C)
    lab = np.asarray(labels_onehot, np.float32)
    w = np.asarray(weights, np.float32)
    avg = np.asarray(average_features, np.float32)

    l_img = np.argmax(lab, axis=1)
    lp = np.repeat(l_img, P)                                    # [23040]
    an = avg / np.maximum(np.linalg.norm(avg, axis=1, keepdims=True), EPS)

    npad = NCORES * R
    zp = np.zeros((npad, C), np.float32)
    zp[:ROWS] = z
    meta = np.zeros((npad, 8), np.float32)
    meta[:ROWS, 0:4] = np.eye(C, dtype=np.float32)[lp]
    meta[:ROWS, 4] = w[lp]
    meta[:ROWS, 5] = (lp > 0).astype(np.float32)
    meta[:ROWS, 6] = w[0]

    avgtn = np.ascontiguousarray(
        an.T.reshape(K, 128, C).transpose(1, 0, 2).reshape(128, K * C))
    eye5 = np.eye(5, dtype=np.float32)

    in_maps = []
    for ci in range(NCORES):
        lo, hi = ci * R, (ci + 1) * R
        fslice = feats[lo:min(hi, ROWS)]
        ft = np.zeros((D, R), np.float32)
        ft[:, :fslice.shape[0]] = fslice.T
        zrow = np.ascontiguousarray(
            zp[lo:hi].reshape(RT, 128, C).transpose(1, 0, 2).reshape(128, RT * C))
        metar = np.ascontiguousarray(
            meta[lo:hi].reshape(RT, 128, 8).transpose(1, 0, 2).reshape(128, RT * 8))
        in_maps.append({"featt": np.ascontiguousarray(ft), "avgtn": avgtn,
                        "zrow": zrow, "meta": metar, "eye5": eye5})
    return in_maps


def kernel(features, average_features, outputs, labels_onehot, weights,
           _trace=False, _trace_kwargs=None):
    if "nc" not in _CACHE:
        _CACHE["nc"] = _build()
    nc = _CACHE["nc"]
    in_maps = _prep(features, average_features, outputs, labels_onehot, weights)
    kwargs = {}
    if _trace:
        kwargs = dict(trace=True, **(_trace_kwargs or {}))
    res = run_bass_kernel_spmd(nc, in_maps, core_ids=list(range(NCORES)), **kwargs)
    total = np.float64(0.0)
    for r in res.results:
        total += np.float64(r["out"].sum())
    _CACHE["last_results"] = res
    return np.float32(total / ROWS)
